# revision 19
# baseline (speedup 1.0000x reference)
"""Multi-head self-attention (B=16, N=1024, D=768, H=12) on 8 TRN2 NeuronCores.

Data-parallel over batch (2 batches per core, weights replicated, no
collectives). Per core, one fused Bass/Tile kernel:

  x --one contiguous interleaved DMA--> x6 [128, 8*768] (token 8p+t on
      partition p, slot t; attention is permutation-invariant over tokens,
      so the interleave is only undone at the output DMA)
  x6 --f16 cast + PE transpose--> xT [d, tok]
  QT/KT = (W_qkv^T x^T + b) in [col, tok] layout (f16)
  V_aug = [x W_v | ones-col per head]  [tok, 12*65] (f16)
  per head: S^T[m,n] = K Q^T (PE), E = exp(S^T*scale) (ACT, [128,1024]),
      O^T = V_aug^T E (PE; row 64 = softmax denominator via the ones
      column -- no max subtraction needed, scores are O(1)).
      normalize: copy O^T/denom out of PSUM early, recip_approx_fast,
      DMA-broadcast the reciprocal row, one DVE mul.
  out = attnT^T W_proj + (W_proj^T b_v + b_proj)  (rank-1 bias matmul;
      V-bias folded through softmax since rows of A sum to 1)

All matmul operands f16 (1 cycle/row; fp32/f32r run 2-pass fp32_mode=HIGH
at 1/4 rate and break HAM warm-up -- measured). PSUM accumulation is f32.
"""

import numpy as np

_CACHE: dict = {}

P = 128
BL, N, D, H, HD = 2, 1024, 768, 12, 64
D3 = 3 * D
SCALE = float(HD) ** -0.5


def _build():
    import concourse.mybir as mybir
    import concourse.tile as tile
    from concourse import bacc
    from concourse.masks import make_identity

    dt = mybir.dt
    F32, F16 = dt.float32, dt.float16
    AF = mybir.ActivationFunctionType

    nc = bacc.Bacc("TRN2", target_bir_lowering=False, debug=False)
    x_d = nc.dram_tensor("x", [BL, N, D], F32, kind="ExternalInput").ap()
    wqkv_d = nc.dram_tensor("W_qkv", [D, D3], F32, kind="ExternalInput").ap()
    bqkv_d = nc.dram_tensor("b_qkv", [D3], F32, kind="ExternalInput").ap()
    wproj_d = nc.dram_tensor("W_proj", [D, D], F32, kind="ExternalInput").ap()
    bproj_d = nc.dram_tensor("b_proj", [D], F32, kind="ExternalInput").ap()
    out_d = nc.dram_tensor("out", [BL, N, D], F32, kind="ExternalOutput").ap()
    # token-interleaved views: partition p, slot t <-> token 8p+t
    x_il = x_d.rearrange("b (p i) d -> b p (i d)", p=P)       # [2, 128, 6144]
    out_il = out_d.rearrange("b (p i) d -> b i p d", p=P)     # [2, 8, 128, 768]

    with tile.TileContext(nc) as tc:
        with tc.tile_pool(name="sb", bufs=1) as sb, \
             tc.tile_pool(name="dp", bufs=1, space="DRAM") as dp, \
             tc.tile_pool(name="ps", bufs=2, space="PSUM") as ps:

            # ---------- constants ----------
            ident = sb.tile([P, P], F16, tag="ident", bufs=1, name="ident")
            make_identity(nc, ident[:])
            ones_h = sb.tile([P, P], F16, tag="ones_h", bufs=1, name="ones_h")
            nc.vector.memset(ones_h[:], 1.0)

            # ---------- x load: one big interleaved DMA per batch ----------
            x6 = {}
            x6[0] = sb.tile([P, 8 * D], F32, tag="x6", bufs=1, name="x6")
            nc.sync.dma_start(x6[0][:, 0:4 * D], x_il[0][:, 0:4 * D])
            nc.sync.dma_start(x6[0][:, 4 * D:8 * D], x_il[0][:, 4 * D:8 * D])

            # ---------- W DMAs on the (idle) scalar queue, casts on DVE ----
            HW_ = D3 // 2
            wq_h, wp_h = [], []
            for d in range(6):
                t = sb.tile([P, D3], F16, tag=f"wqkv{d}", bufs=1, name=f"wqkv{d}")
                for half in range(2):
                    stg = sb.tile([P, HW_], F32, tag="wstage", bufs=3, name="wstg")
                    nc.scalar.dma_start(
                        stg[:], wqkv_d[P * d:P * (d + 1), HW_ * half:HW_ * (half + 1)])
                    nc.vector.tensor_copy(t[:, HW_ * half:HW_ * (half + 1)], stg[:])
                wq_h.append(t)

            xT = {b: [sb.tile([P, N], F16, tag=f"xT{b}_{j}", bufs=1,
                              name=f"xT{b}_{j}") for j in range(6)]
                  for b in range(BL)}

            def do_transposes(b):
                for t in range(8):
                    xh = sb.tile([P, D], F16, tag="xh", bufs=2, name="xh")
                    nc.vector.tensor_copy(xh[:], x6[b][:, D * t:D * (t + 1)])
                    for j in range(6):
                        tp = ps.tile([P, P], F16, tag="mm", bufs=2, name="tp")
                        nc.tensor.transpose(tp[:], xh[:, P * j:P * (j + 1)],
                                            ident[:])
                        nc.vector.tensor_copy(xT[b][j][:, P * t:P * (t + 1)],
                                              tp[:])

            do_transposes(0)

            for d in range(6):
                stg = sb.tile([P, HW_], F32, tag="wstage", bufs=3, name="wstg2")
                nc.scalar.dma_start(stg[:, 0:D], wproj_d[P * d:P * (d + 1), :])
                t = sb.tile([P, D], F16, tag=f"wproj{d}", bufs=1, name=f"wproj{d}")
                nc.vector.tensor_copy(t[:], stg[:, 0:D])
                wp_h.append(t)

            # ---------- biases (one [18,128] DMA + PE transpose) ----------
            bstg = sb.tile([18, P], F32, tag="bstg", bufs=1, name="bstg")
            nc.scalar.dma_start(bstg[:], bqkv_d.rearrange("(j p) -> j p", p=P))
            bstg_h = sb.tile([18, P], F16, tag="bstg_h", bufs=1, name="bstg_h")
            nc.vector.tensor_copy(bstg_h[:], bstg[:])
            btp = ps.tile([P, 18], F16, tag="mm", bufs=2, name="btp")
            nc.tensor.transpose(btp[:], bstg_h[:], ident[0:18, 0:18])
            bqkvT = sb.tile([P, 18], F32, tag="bqkvT", bufs=1, name="bqkvT")
            nc.vector.tensor_copy(bqkvT[:], btp[:])
            bv_h = sb.tile([P, 6], F16, tag="bv_h", bufs=1, name="bv_h")
            nc.vector.tensor_copy(bv_h[:], btp[:, 12:18])
            bproj_row = sb.tile([1, D], F32, tag="bproj_row", bufs=1, name="bproj_row")
            nc.scalar.dma_start(bproj_row[:], bproj_d.unsqueeze(0))

            # b_final = W_proj^T b_v + b_proj   [1, 768] f16
            bfinal_h = sb.tile([1, D], F16, tag="bfinal", bufs=1, name="bfinal")
            for c0, cw in ((0, 512), (512, 256)):
                bf_ps = ps.tile([1, 512], F32, tag="mm", bufs=2, name="bf_ps")
                for d in range(6):
                    nc.tensor.matmul(bf_ps[:, 0:cw], bv_h[:, d:d + 1],
                                     wp_h[d][:, c0:c0 + cw],
                                     start=(d == 0), stop=(d == 5))
                nc.vector.tensor_add(bfinal_h[:, c0:c0 + cw], bf_ps[0:1, 0:cw],
                                     bproj_row[:, c0:c0 + cw])

            # ---------- per-batch ----------
            for b in range(BL):
                xTb = xT[b]

                # QT/KT: [col_tile][128 cols, 1024 toks] f16, bias added
                qk = [sb.tile([P, N], F16, tag=f"qk{j}", bufs=2, name=f"qk{j}")
                      for j in range(12)]
                for j in range(12):
                    for nh in range(2):
                        qps = ps.tile([P, 512], F32, tag="mm", bufs=2, name="qps")
                        for d in range(6):
                            nc.tensor.matmul(qps[:], wq_h[d][:, P * j:P * (j + 1)],
                                             xTb[d][:, 512 * nh:512 * (nh + 1)],
                                             start=(d == 0), stop=(d == 5))
                        nc.vector.tensor_scalar_add(
                            qk[j][:, 512 * nh:512 * (nh + 1)], qps[:], bqkvT[:, j:j + 1])

                # V_aug: [tok_tile][128, 12*65] f16 (col 64 of each head = 1)
                v = [sb.tile([P, 12 * 65], F16, tag=f"v{t}", bufs=1, name=f"v{t}")
                     for t in range(8)]
                for t in range(8):
                    v3 = v[t].rearrange("p (h c) -> p h c", c=65)
                    nc.vector.tensor_copy(v3[:, :, 64:65],
                                          ones_h[:, 0:12].unsqueeze(2))
                    for c0, cw in ((0, 512), (512, 256)):
                        vps = ps.tile([P, 512], F32, tag="mm", bufs=2, name="vps")
                        for d in range(6):
                            nc.tensor.matmul(vps[:, 0:cw], xTb[d][:, P * t:P * (t + 1)],
                                             wq_h[d][:, 2 * D + c0:2 * D + c0 + cw],
                                             start=(d == 0), stop=(d == 5))
                        nc.vector.tensor_copy(
                            v3[:, (c0 // HD):((c0 + cw) // HD), 0:HD],
                            vps[:, 0:cw].rearrange("p (h c) -> p h c", c=HD))

                # prefetch next batch's x + transposes (fills PE slack
                # during this batch's attention)
                if b + 1 < BL:
                    x6[b + 1] = sb.tile([P, 8 * D], F32, tag="x6", bufs=1,
                                        name="x6")
                    nc.sync.dma_start(x6[b + 1][:, 0:4 * D],
                                      x_il[b + 1][:, 0:4 * D])
                    nc.sync.dma_start(x6[b + 1][:, 4 * D:8 * D],
                                      x_il[b + 1][:, 4 * D:8 * D])
                    do_transposes(b + 1)

                # attention: head pairs -- the two K=64 score matmuls run
                # concurrently in PE row groups, writing the two banks of one
                # S psum tile; one 1024-wide exp covers both heads.
                at = [sb.tile([P, N], F16, tag=f"at{j}", bufs=1, name=f"at{j}")
                      for j in range(6)]
                for jp in range(6):
                    qt, kt = qk[jp], qk[6 + jp]
                    for nh in range(2):
                        n0 = 512 * nh
                        ot = [ps.tile([65, 512], F32, tag="ot", bufs=2,
                                      name="otps") for _ in range(2)]
                        for m in range(8):
                            sps = ps.tile([P, N], F32, tag="s", bufs=2,
                                          name="sps")
                            for hh in range(2):
                                r0, r1 = HD * hh, HD * (hh + 1)
                                nc.tensor.matmul(sps[:, 512 * hh:512 * (hh + 1)],
                                                 kt[r0:r1, P * m:P * (m + 1)],
                                                 qt[r0:r1, n0:n0 + 512],
                                                 start=True, stop=True)
                            e = sb.tile([P, N], F16, tag="e", bufs=4, name="e")
                            nc.scalar.activation(e[:], sps[:], AF.Exp,
                                                 scale=SCALE)
                            for hh in range(2):
                                h = 2 * jp + hh
                                nc.tensor.matmul(ot[hh][:],
                                                 v[m][:, 65 * h:65 * h + 65],
                                                 e[:, 512 * hh:512 * (hh + 1)],
                                                 start=(m == 0), stop=(m == 7))
                        for hh in range(2):
                            r0, r1 = HD * hh, HD * (hh + 1)
                            u_sb = sb.tile([HD, 512], F16, tag="u_sb", bufs=3,
                                           name="u_sb")
                            nc.vector.tensor_copy(u_sb[:], ot[hh][0:HD, :])
                            dr_f = sb.tile([1, 512], F32, tag="dr_f", bufs=2,
                                           name="dr_f")
                            nc.vector.tensor_copy(dr_f[:], ot[hh][64:65, :])
                            rr_f = sb.tile([1, 512], F32, tag="rr_f", bufs=2,
                                           name="rr_f")
                            nc.vector.reciprocal_approx_fast(out=rr_f[:],
                                                             in_=dr_f[:])
                            rr_h = sb.tile([1, 512], F16, tag="rr_h", bufs=2,
                                           name="rr_h")
                            nc.gpsimd.tensor_copy(rr_h[:], rr_f[:])
                            rr_d = dp.tile([1, 512], F16, tag="rr_d", bufs=2,
                                           name="rr_d")
                            nc.sync.dma_start(rr_d[:], rr_h[:])
                            bc_h = sb.tile([HD, 512], F16, tag="bc_h", bufs=2,
                                           name="bc_h")
                            nc.sync.dma_start(bc_h[:],
                                              rr_d[:].to_broadcast((HD, 512)))
                            nc.vector.tensor_mul(at[jp][r0:r1, n0:n0 + 512],
                                                 u_sb[:], bc_h[:])

                # output projection (+ rank-1 bias), psum -> sbuf -> DRAM
                for t in range(8):
                    osb = sb.tile([P, D], F32, tag="outs", bufs=2, name="osb")
                    for c0, cw in ((0, 512), (512, 256)):
                        pps = ps.tile([P, 512], F32, tag="mm", bufs=2, name="pps")
                        for d in range(6):
                            nc.tensor.matmul(pps[:, 0:cw], at[d][:, P * t:P * (t + 1)],
                                             wp_h[d][:, c0:c0 + cw],
                                             start=(d == 0), stop=False)
                        nc.tensor.matmul(pps[:, 0:cw], ones_h[0:1, 0:P],
                                         bfinal_h[:, c0:c0 + cw],
                                         start=False, stop=True)
                        nc.vector.tensor_copy(osb[:, c0:c0 + cw], pps[:, 0:cw])
                    nc.sync.dma_start(out_il[b, t], osb[:])
    nc.compile()
    return nc


def _get_nc():
    if "nc" not in _CACHE:
        _CACHE["nc"] = _build()
    return _CACHE["nc"]


def kernel(x, W_qkv, b_qkv, W_proj, b_proj):
    from concourse.bass_utils import run_bass_kernel_spmd

    nc = _get_nc()
    x = np.ascontiguousarray(x, dtype=np.float32)
    in_maps = [
        {
            "x": x[2 * i:2 * i + 2],
            "W_qkv": np.asarray(W_qkv, dtype=np.float32),
            "b_qkv": np.asarray(b_qkv, dtype=np.float32),
            "W_proj": np.asarray(W_proj, dtype=np.float32),
            "b_proj": np.asarray(b_proj, dtype=np.float32),
        }
        for i in range(8)
    ]
    res = run_bass_kernel_spmd(nc, in_maps, core_ids=list(range(8)))
    return np.concatenate([r["out"] for r in res.results], axis=0)


# revision 20
# speedup vs baseline: 1.0732x; 1.0732x over previous
"""Multi-head self-attention (B=16, N=1024, D=768, H=12) on 8 TRN2 NeuronCores.

Data-parallel over batch (2 batches per core, weights replicated, no
collectives). Per core, one fused Bass/Tile kernel:

  x --one contiguous interleaved DMA--> x6 [128, 8*768] (token 8p+t on
      partition p, slot t; attention is permutation-invariant over tokens,
      so the interleave is only undone at the output DMA)
  x6 --f16 cast + PE transpose--> xT [d, tok]
  QT/KT = (W_qkv^T x^T + b) in [col, tok] layout (f16)
  V_aug = [x W_v | ones-col per head]  [tok, 12*65] (f16)
  per head: S^T[m,n] = K Q^T (PE), E = exp(S^T*scale) (ACT, [128,1024]),
      O^T = V_aug^T E (PE; row 64 = softmax denominator via the ones
      column -- no max subtraction needed, scores are O(1)).
      normalize: copy O^T/denom out of PSUM early, recip_approx_fast,
      DMA-broadcast the reciprocal row, one DVE mul.
  out = attnT^T W_proj + (W_proj^T b_v + b_proj)  (rank-1 bias matmul;
      V-bias folded through softmax since rows of A sum to 1)

All matmul operands f16 (1 cycle/row; fp32/f32r run 2-pass fp32_mode=HIGH
at 1/4 rate and break HAM warm-up -- measured). PSUM accumulation is f32.
"""

import numpy as np

_CACHE: dict = {}

P = 128
BL, N, D, H, HD = 2, 1024, 768, 12, 64
D3 = 3 * D
SCALE = float(HD) ** -0.5


def _build():
    import concourse.mybir as mybir
    import concourse.tile as tile
    from concourse import bacc
    from concourse.masks import make_identity

    dt = mybir.dt
    F32, F16 = dt.float32, dt.float16
    AF = mybir.ActivationFunctionType

    nc = bacc.Bacc("TRN2", target_bir_lowering=False, debug=False)
    x_d = nc.dram_tensor("x", [BL, N, D], F32, kind="ExternalInput").ap()
    wqkv_d = nc.dram_tensor("W_qkv", [D, D3], F32, kind="ExternalInput").ap()
    bqkv_d = nc.dram_tensor("b_qkv", [D3], F32, kind="ExternalInput").ap()
    wproj_d = nc.dram_tensor("W_proj", [D, D], F32, kind="ExternalInput").ap()
    bproj_d = nc.dram_tensor("b_proj", [D], F32, kind="ExternalInput").ap()
    out_d = nc.dram_tensor("out", [BL, N, D], F32, kind="ExternalOutput").ap()
    # token-interleaved views: partition p, slot t <-> token 8p+t
    x_il = x_d.rearrange("b (p i) d -> b p (i d)", p=P)       # [2, 128, 6144]
    out_il = out_d.rearrange("b (p i) d -> b i p d", p=P)     # [2, 8, 128, 768]

    with tile.TileContext(nc) as tc:
        with tc.tile_pool(name="sb", bufs=1) as sb, \
             tc.tile_pool(name="dp", bufs=1, space="DRAM") as dp, \
             tc.tile_pool(name="ps", bufs=2, space="PSUM") as ps:

            # ---------- constants ----------
            ident = sb.tile([P, P], F16, tag="ident", bufs=1, name="ident")
            make_identity(nc, ident[:])
            ones_h = sb.tile([P, P], F16, tag="ones_h", bufs=1, name="ones_h")
            nc.vector.memset(ones_h[:], 1.0)

            # ---------- x load: one big interleaved DMA per batch ----------
            x6 = {}
            x6[0] = sb.tile([P, 8 * D], F32, tag="x6", bufs=1, name="x6")
            nc.sync.dma_start(x6[0][:, 0:4 * D], x_il[0][:, 0:4 * D])
            nc.sync.dma_start(x6[0][:, 4 * D:8 * D], x_il[0][:, 4 * D:8 * D])

            # ---------- W DMAs on the (idle) scalar queue, casts on DVE ----
            HW_ = D3 // 2
            wq_h, wp_h = [], []
            for d in range(6):
                t = sb.tile([P, D3], F16, tag=f"wqkv{d}", bufs=1, name=f"wqkv{d}")
                for half in range(2):
                    stg = sb.tile([P, HW_], F32, tag="wstage", bufs=3, name="wstg")
                    nc.scalar.dma_start(
                        stg[:], wqkv_d[P * d:P * (d + 1), HW_ * half:HW_ * (half + 1)])
                    nc.vector.tensor_copy(t[:, HW_ * half:HW_ * (half + 1)], stg[:])
                wq_h.append(t)

            xT = {b: [sb.tile([P, N], F16, tag=f"xT{b}_{j}", bufs=1,
                              name=f"xT{b}_{j}") for j in range(6)]
                  for b in range(BL)}

            def do_transposes(b):
                for t in range(8):
                    xh = sb.tile([P, D], F16, tag="xh", bufs=2, name="xh")
                    nc.vector.tensor_copy(xh[:], x6[b][:, D * t:D * (t + 1)])
                    for j in range(6):
                        tp = ps.tile([P, P], F16, tag="mm", bufs=2, name="tp")
                        nc.tensor.transpose(tp[:], xh[:, P * j:P * (j + 1)],
                                            ident[:])
                        nc.vector.tensor_copy(xT[b][j][:, P * t:P * (t + 1)],
                                              tp[:])

            do_transposes(0)

            for d in range(6):
                stg = sb.tile([P, HW_], F32, tag="wstage", bufs=3, name="wstg2")
                nc.scalar.dma_start(stg[:, 0:D], wproj_d[P * d:P * (d + 1), :])
                t = sb.tile([P, D], F16, tag=f"wproj{d}", bufs=1, name=f"wproj{d}")
                nc.vector.tensor_copy(t[:], stg[:, 0:D])
                wp_h.append(t)

            # ---------- biases (one [18,128] DMA + PE transpose) ----------
            bstg = sb.tile([18, P], F32, tag="bstg", bufs=1, name="bstg")
            nc.scalar.dma_start(bstg[:], bqkv_d.rearrange("(j p) -> j p", p=P))
            bstg_h = sb.tile([18, P], F16, tag="bstg_h", bufs=1, name="bstg_h")
            nc.vector.tensor_copy(bstg_h[:], bstg[:])
            btp = ps.tile([P, 18], F16, tag="mm", bufs=2, name="btp")
            nc.tensor.transpose(btp[:], bstg_h[:], ident[0:18, 0:18])
            bqkvT = sb.tile([P, 18], F32, tag="bqkvT", bufs=1, name="bqkvT")
            nc.vector.tensor_copy(bqkvT[:], btp[:])
            bv_h = sb.tile([P, 6], F16, tag="bv_h", bufs=1, name="bv_h")
            nc.vector.tensor_copy(bv_h[:], btp[:, 12:18])
            bproj_row = sb.tile([1, D], F32, tag="bproj_row", bufs=1, name="bproj_row")
            nc.scalar.dma_start(bproj_row[:], bproj_d.unsqueeze(0))

            # b_final = W_proj^T b_v + b_proj   [1, 768] f16
            bfinal_h = sb.tile([1, D], F16, tag="bfinal", bufs=1, name="bfinal")
            for c0, cw in ((0, 512), (512, 256)):
                bf_ps = ps.tile([1, 512], F32, tag="mm", bufs=2, name="bf_ps")
                for d in range(6):
                    nc.tensor.matmul(bf_ps[:, 0:cw], bv_h[:, d:d + 1],
                                     wp_h[d][:, c0:c0 + cw],
                                     start=(d == 0), stop=(d == 5))
                nc.vector.tensor_add(bfinal_h[:, c0:c0 + cw], bf_ps[0:1, 0:cw],
                                     bproj_row[:, c0:c0 + cw])

            # ---------- per-batch ----------
            for b in range(BL):
                xTb = xT[b]

                # QT/KT: [col_tile][128 cols, 1024 toks] f16, bias added
                qk = [sb.tile([P, N], F16, tag=f"qk{j}", bufs=2, name=f"qk{j}")
                      for j in range(12)]
                for j in range(12):
                    for nh in range(2):
                        qps = ps.tile([P, 512], F32, tag="mm", bufs=2, name="qps")
                        for d in range(6):
                            nc.tensor.matmul(qps[:], wq_h[d][:, P * j:P * (j + 1)],
                                             xTb[d][:, 512 * nh:512 * (nh + 1)],
                                             start=(d == 0), stop=(d == 5))
                        nc.vector.tensor_scalar_add(
                            qk[j][:, 512 * nh:512 * (nh + 1)], qps[:], bqkvT[:, j:j + 1])

                # V_aug: [tok_tile][128, 12*65] f16 (col 64 of each head = 1)
                v = [sb.tile([P, 12 * 65], F16, tag=f"v{t}", bufs=1, name=f"v{t}")
                     for t in range(8)]
                for t in range(8):
                    v3 = v[t].rearrange("p (h c) -> p h c", c=65)
                    nc.vector.tensor_copy(v3[:, :, 64:65],
                                          ones_h[:, 0:12].unsqueeze(2))
                    for c0, cw in ((0, 512), (512, 256)):
                        vps = ps.tile([P, 512], F32, tag="mm", bufs=2, name="vps")
                        for d in range(6):
                            nc.tensor.matmul(vps[:, 0:cw], xTb[d][:, P * t:P * (t + 1)],
                                             wq_h[d][:, 2 * D + c0:2 * D + c0 + cw],
                                             start=(d == 0), stop=(d == 5))
                        nc.vector.tensor_copy(
                            v3[:, (c0 // HD):((c0 + cw) // HD), 0:HD],
                            vps[:, 0:cw].rearrange("p (h c) -> p h c", c=HD))

                # prefetch next batch's x + transposes (fills PE slack
                # during this batch's attention)
                if b + 1 < BL:
                    x6[b + 1] = sb.tile([P, 8 * D], F32, tag="x6", bufs=1,
                                        name="x6")
                    nc.sync.dma_start(x6[b + 1][:, 0:4 * D],
                                      x_il[b + 1][:, 0:4 * D])
                    nc.sync.dma_start(x6[b + 1][:, 4 * D:8 * D],
                                      x_il[b + 1][:, 4 * D:8 * D])
                    do_transposes(b + 1)

                # attention: head pairs -- the two K=64 score matmuls run
                # concurrently in PE row groups, writing the two banks of one
                # S psum tile; one 1024-wide exp covers both heads.
                at = [sb.tile([P, N], F16, tag=f"at{j}", bufs=1, name=f"at{j}")
                      for j in range(6)]
                for jp in range(6):
                    qt, kt = qk[jp], qk[6 + jp]
                    for nh in range(2):
                        n0 = 512 * nh
                        ot = [ps.tile([65, 512], F32, tag="ot", bufs=2,
                                      name="otps") for _ in range(2)]
                        for m in range(8):
                            sps = ps.tile([P, N], F32, tag="s", bufs=2,
                                          name="sps")
                            for hh in range(2):
                                r0, r1 = HD * hh, HD * (hh + 1)
                                nc.tensor.matmul(sps[:, 512 * hh:512 * (hh + 1)],
                                                 kt[r0:r1, P * m:P * (m + 1)],
                                                 qt[r0:r1, n0:n0 + 512],
                                                 start=True, stop=True)
                            e = sb.tile([P, N], F16, tag="e", bufs=3, name="e")
                            nc.scalar.activation(e[:], sps[:], AF.Exp,
                                                 scale=SCALE)
                            for hh in range(2):
                                h = 2 * jp + hh
                                nc.tensor.matmul(ot[hh][:],
                                                 v[m][:, 65 * h:65 * h + 65],
                                                 e[:, 512 * hh:512 * (hh + 1)],
                                                 start=(m == 0), stop=(m == 7))
                        for hh in range(2):
                            r0, r1 = HD * hh, HD * (hh + 1)
                            u_sb = sb.tile([HD, 512], F16, tag="u_sb", bufs=3,
                                           name="u_sb")
                            nc.vector.tensor_copy(u_sb[:], ot[hh][0:HD, :])
                            dr_f = sb.tile([1, 512], F32, tag="dr_f", bufs=2,
                                           name="dr_f")
                            nc.vector.tensor_copy(dr_f[:], ot[hh][64:65, :])
                            rr_f = sb.tile([1, 512], F32, tag="rr_f", bufs=2,
                                           name="rr_f")
                            nc.vector.reciprocal_approx_fast(out=rr_f[:],
                                                             in_=dr_f[:])
                            rr_h = sb.tile([1, 512], F16, tag="rr_h", bufs=2,
                                           name="rr_h")
                            nc.vector.tensor_copy(rr_h[:], rr_f[:])
                            rr_d = dp.tile([1, 512], F16, tag="rr_d", bufs=2,
                                           name="rr_d")
                            nc.sync.dma_start(rr_d[:], rr_h[:])
                            bc_h = sb.tile([HD, 512], F16, tag="bc_h", bufs=2,
                                           name="bc_h")
                            nc.sync.dma_start(bc_h[:],
                                              rr_d[:].to_broadcast((HD, 512)))
                            nc.vector.tensor_mul(at[jp][r0:r1, n0:n0 + 512],
                                                 u_sb[:], bc_h[:])

                # output projection (+ rank-1 bias), psum -> sbuf -> DRAM
                for t in range(8):
                    osb = sb.tile([P, D], F32, tag="outs", bufs=2, name="osb")
                    for c0, cw in ((0, 512), (512, 256)):
                        pps = ps.tile([P, 512], F32, tag="mm", bufs=2, name="pps")
                        for d in range(6):
                            nc.tensor.matmul(pps[:, 0:cw], at[d][:, P * t:P * (t + 1)],
                                             wp_h[d][:, c0:c0 + cw],
                                             start=(d == 0), stop=False)
                        nc.tensor.matmul(pps[:, 0:cw], ones_h[0:1, 0:P],
                                         bfinal_h[:, c0:c0 + cw],
                                         start=False, stop=True)
                        nc.vector.tensor_copy(osb[:, c0:c0 + cw], pps[:, 0:cw])
                    nc.sync.dma_start(out_il[b, t], osb[:])
    nc.compile()
    return nc


def _get_nc():
    if "nc" not in _CACHE:
        _CACHE["nc"] = _build()
    return _CACHE["nc"]


def kernel(x, W_qkv, b_qkv, W_proj, b_proj):
    from concourse.bass_utils import run_bass_kernel_spmd

    nc = _get_nc()
    x = np.ascontiguousarray(x, dtype=np.float32)
    in_maps = [
        {
            "x": x[2 * i:2 * i + 2],
            "W_qkv": np.asarray(W_qkv, dtype=np.float32),
            "b_qkv": np.asarray(b_qkv, dtype=np.float32),
            "W_proj": np.asarray(W_proj, dtype=np.float32),
            "b_proj": np.asarray(b_proj, dtype=np.float32),
        }
        for i in range(8)
    ]
    res = run_bass_kernel_spmd(nc, in_maps, core_ids=list(range(8)))
    return np.concatenate([r["out"] for r in res.results], axis=0)


# revision 22
# speedup vs baseline: 1.0789x; 1.0053x over previous
"""Multi-head self-attention (B=16, N=1024, D=768, H=12) on 8 TRN2 NeuronCores.

Data-parallel over batch (2 batches per core, weights replicated, no
collectives). Per core, one fused Bass/Tile kernel:

  x --one contiguous interleaved DMA--> x6 [128, 8*768] (token 8p+t on
      partition p, slot t; attention is permutation-invariant over tokens,
      so the interleave is only undone at the output DMA)
  x6 --f16 cast + PE transpose--> xT [d, tok]
  QT/KT = (W_qkv^T x^T + b) in [col, tok] layout (f16)
  V_aug = [x W_v | ones-col per head]  [tok, 12*65] (f16)
  per head: S^T[m,n] = K Q^T (PE), E = exp(S^T*scale) (ACT, [128,1024]),
      O^T = V_aug^T E (PE; row 64 = softmax denominator via the ones
      column -- no max subtraction needed, scores are O(1)).
      normalize: copy O^T/denom out of PSUM early, recip_approx_fast,
      DMA-broadcast the reciprocal row, one DVE mul.
  out = attnT^T W_proj + (W_proj^T b_v + b_proj)  (rank-1 bias matmul;
      V-bias folded through softmax since rows of A sum to 1)

All matmul operands f16 (1 cycle/row; fp32/f32r run 2-pass fp32_mode=HIGH
at 1/4 rate and break HAM warm-up -- measured). PSUM accumulation is f32.
"""

import numpy as np

_CACHE: dict = {}

P = 128
BL, N, D, H, HD = 2, 1024, 768, 12, 64
D3 = 3 * D
SCALE = float(HD) ** -0.5


def _build():
    import concourse.mybir as mybir
    import concourse.tile as tile
    from concourse import bacc
    from concourse.masks import make_identity

    dt = mybir.dt
    F32, F16 = dt.float32, dt.float16
    AF = mybir.ActivationFunctionType

    nc = bacc.Bacc("TRN2", target_bir_lowering=False, debug=False)
    x_d = nc.dram_tensor("x", [BL, N, D], F32, kind="ExternalInput").ap()
    wqkv_d = nc.dram_tensor("W_qkv", [D, D3], F32, kind="ExternalInput").ap()
    bqkv_d = nc.dram_tensor("b_qkv", [D3], F32, kind="ExternalInput").ap()
    wproj_d = nc.dram_tensor("W_proj", [D, D], F32, kind="ExternalInput").ap()
    bproj_d = nc.dram_tensor("b_proj", [D], F32, kind="ExternalInput").ap()
    out_d = nc.dram_tensor("out", [BL, N, D], F32, kind="ExternalOutput").ap()
    # token-interleaved views: partition p, slot t <-> token 8p+t
    x_il = x_d.rearrange("b (p i) d -> b p (i d)", p=P)       # [2, 128, 6144]
    out_il = out_d.rearrange("b (p i) d -> b i p d", p=P)     # [2, 8, 128, 768]

    with tile.TileContext(nc) as tc:
        with tc.tile_pool(name="sb", bufs=1) as sb, \
             tc.tile_pool(name="dp", bufs=1, space="DRAM") as dp, \
             tc.tile_pool(name="ps", bufs=2, space="PSUM") as ps:

            # ---------- constants ----------
            ident = sb.tile([P, P], F16, tag="ident", bufs=1, name="ident")
            make_identity(nc, ident[:])
            ones_h = sb.tile([P, P], F16, tag="ones_h", bufs=1, name="ones_h")
            nc.vector.memset(ones_h[:], 1.0)

            # ---------- PE warm-up: ~4us of dummy matmuls flips HAM to 8/8
            # (transposes run in transpose-mode, which does not warm HAM)
            warm_h = sb.tile([P, 512], F16, tag="warm_h", bufs=1, name="warm_h")
            nc.vector.memset(warm_h[:], 0.0)
            for wi in range(10):
                wps = ps.tile([P, 512], F32, tag="mm", bufs=2, name="wps")
                nc.tensor.matmul(wps[:], ones_h[:, 0:P], warm_h[:],
                                 start=True, stop=True)

            # ---------- x load: one big interleaved DMA per batch ----------
            x6 = {}
            x6[0] = sb.tile([P, 8 * D], F32, tag="x6", bufs=1, name="x6")
            nc.sync.dma_start(x6[0][:, 0:4 * D], x_il[0][:, 0:4 * D])
            nc.sync.dma_start(x6[0][:, 4 * D:8 * D], x_il[0][:, 4 * D:8 * D])

            # ---------- W DMAs on the (idle) scalar queue, casts on DVE ----
            HW_ = D3 // 2
            wq_h, wp_h = [], []
            for d in range(6):
                t = sb.tile([P, D3], F16, tag=f"wqkv{d}", bufs=1, name=f"wqkv{d}")
                for half in range(2):
                    stg = sb.tile([P, HW_], F32, tag="wstage", bufs=3, name="wstg")
                    nc.scalar.dma_start(
                        stg[:], wqkv_d[P * d:P * (d + 1), HW_ * half:HW_ * (half + 1)])
                    nc.vector.tensor_copy(t[:, HW_ * half:HW_ * (half + 1)], stg[:])
                wq_h.append(t)

            xT = {b: [sb.tile([P, N], F16, tag=f"xT{b}_{j}", bufs=1,
                              name=f"xT{b}_{j}") for j in range(6)]
                  for b in range(BL)}

            def do_transposes(b):
                for t in range(8):
                    xh = sb.tile([P, D], F16, tag="xh", bufs=2, name="xh")
                    nc.vector.tensor_copy(xh[:], x6[b][:, D * t:D * (t + 1)])
                    for j in range(6):
                        tp = ps.tile([P, P], F16, tag="mm", bufs=2, name="tp")
                        nc.tensor.transpose(tp[:], xh[:, P * j:P * (j + 1)],
                                            ident[:])
                        nc.vector.tensor_copy(xT[b][j][:, P * t:P * (t + 1)],
                                              tp[:])

            do_transposes(0)

            for d in range(6):
                stg = sb.tile([P, HW_], F32, tag="wstage", bufs=3, name="wstg2")
                nc.scalar.dma_start(stg[:, 0:D], wproj_d[P * d:P * (d + 1), :])
                t = sb.tile([P, D], F16, tag=f"wproj{d}", bufs=1, name=f"wproj{d}")
                nc.vector.tensor_copy(t[:], stg[:, 0:D])
                wp_h.append(t)

            # ---------- biases (one [18,128] DMA + PE transpose) ----------
            bstg = sb.tile([18, P], F32, tag="bstg", bufs=1, name="bstg")
            nc.scalar.dma_start(bstg[:], bqkv_d.rearrange("(j p) -> j p", p=P))
            bstg_h = sb.tile([18, P], F16, tag="bstg_h", bufs=1, name="bstg_h")
            nc.vector.tensor_copy(bstg_h[:], bstg[:])
            btp = ps.tile([P, 18], F16, tag="mm", bufs=2, name="btp")
            nc.tensor.transpose(btp[:], bstg_h[:], ident[0:18, 0:18])
            bqkvT = sb.tile([P, 18], F32, tag="bqkvT", bufs=1, name="bqkvT")
            nc.vector.tensor_copy(bqkvT[:], btp[:])
            bv_h = sb.tile([P, 6], F16, tag="bv_h", bufs=1, name="bv_h")
            nc.vector.tensor_copy(bv_h[:], btp[:, 12:18])
            bproj_row = sb.tile([1, D], F32, tag="bproj_row", bufs=1, name="bproj_row")
            nc.scalar.dma_start(bproj_row[:], bproj_d.unsqueeze(0))

            # b_final = W_proj^T b_v + b_proj   [1, 768] f16
            bfinal_h = sb.tile([1, D], F16, tag="bfinal", bufs=1, name="bfinal")
            for c0, cw in ((0, 512), (512, 256)):
                bf_ps = ps.tile([1, 512], F32, tag="mm", bufs=2, name="bf_ps")
                for d in range(6):
                    nc.tensor.matmul(bf_ps[:, 0:cw], bv_h[:, d:d + 1],
                                     wp_h[d][:, c0:c0 + cw],
                                     start=(d == 0), stop=(d == 5))
                nc.vector.tensor_add(bfinal_h[:, c0:c0 + cw], bf_ps[0:1, 0:cw],
                                     bproj_row[:, c0:c0 + cw])

            # ---------- per-batch ----------
            for b in range(BL):
                xTb = xT[b]

                # QT/KT: [col_tile][128 cols, 1024 toks] f16, bias added
                qk = [sb.tile([P, N], F16, tag=f"qk{j}", bufs=2, name=f"qk{j}")
                      for j in range(12)]
                for j in range(12):
                    for nh in range(2):
                        qps = ps.tile([P, 512], F32, tag="mm", bufs=2, name="qps")
                        for d in range(6):
                            nc.tensor.matmul(qps[:], wq_h[d][:, P * j:P * (j + 1)],
                                             xTb[d][:, 512 * nh:512 * (nh + 1)],
                                             start=(d == 0), stop=(d == 5))
                        nc.vector.tensor_scalar_add(
                            qk[j][:, 512 * nh:512 * (nh + 1)], qps[:], bqkvT[:, j:j + 1])

                # V_aug: [tok_tile][128, 12*65] f16 (col 64 of each head = 1)
                v = [sb.tile([P, 12 * 65], F16, tag=f"v{t}", bufs=1, name=f"v{t}")
                     for t in range(8)]
                for t in range(8):
                    v3 = v[t].rearrange("p (h c) -> p h c", c=65)
                    nc.vector.tensor_copy(v3[:, :, 64:65],
                                          ones_h[:, 0:12].unsqueeze(2))
                    for c0, cw in ((0, 512), (512, 256)):
                        vps = ps.tile([P, 512], F32, tag="mm", bufs=2, name="vps")
                        for d in range(6):
                            nc.tensor.matmul(vps[:, 0:cw], xTb[d][:, P * t:P * (t + 1)],
                                             wq_h[d][:, 2 * D + c0:2 * D + c0 + cw],
                                             start=(d == 0), stop=(d == 5))
                        nc.vector.tensor_copy(
                            v3[:, (c0 // HD):((c0 + cw) // HD), 0:HD],
                            vps[:, 0:cw].rearrange("p (h c) -> p h c", c=HD))

                # prefetch next batch's x + transposes (fills PE slack
                # during this batch's attention)
                if b + 1 < BL:
                    x6[b + 1] = sb.tile([P, 8 * D], F32, tag="x6", bufs=1,
                                        name="x6")
                    nc.sync.dma_start(x6[b + 1][:, 0:4 * D],
                                      x_il[b + 1][:, 0:4 * D])
                    nc.sync.dma_start(x6[b + 1][:, 4 * D:8 * D],
                                      x_il[b + 1][:, 4 * D:8 * D])
                    do_transposes(b + 1)

                # attention: head pairs -- the two K=64 score matmuls run
                # concurrently in PE row groups, writing the two banks of one
                # S psum tile; one 1024-wide exp covers both heads.
                at = [sb.tile([P, N], F16, tag=f"at{j}", bufs=1, name=f"at{j}")
                      for j in range(6)]
                for jp in range(6):
                    qt, kt = qk[jp], qk[6 + jp]
                    for nh in range(2):
                        n0 = 512 * nh
                        ot = [ps.tile([65, 512], F32, tag="ot", bufs=2,
                                      name="otps") for _ in range(2)]
                        pend = []
                        for m in range(8):
                            sps = ps.tile([P, N], F32, tag="s", bufs=2,
                                          name="sps")
                            for hh in range(2):
                                r0, r1 = HD * hh, HD * (hh + 1)
                                nc.tensor.matmul(sps[:, 512 * hh:512 * (hh + 1)],
                                                 kt[r0:r1, P * m:P * (m + 1)],
                                                 qt[r0:r1, n0:n0 + 512],
                                                 start=True, stop=True)
                            e = sb.tile([P, N], F16, tag="e", bufs=3, name="e")
                            nc.scalar.activation(e[:], sps[:], AF.Exp,
                                                 scale=SCALE)
                            pend.append((m, e))
                            if len(pend) == 2:
                                pm, pe_ = pend.pop(0)
                                for hh in range(2):
                                    h = 2 * jp + hh
                                    nc.tensor.matmul(
                                        ot[hh][:], v[pm][:, 65 * h:65 * h + 65],
                                        pe_[:, 512 * hh:512 * (hh + 1)],
                                        start=(pm == 0), stop=(pm == 7))
                        for pm, pe_ in pend:
                            for hh in range(2):
                                h = 2 * jp + hh
                                nc.tensor.matmul(
                                    ot[hh][:], v[pm][:, 65 * h:65 * h + 65],
                                    pe_[:, 512 * hh:512 * (hh + 1)],
                                    start=(pm == 0), stop=(pm == 7))
                        for hh in range(2):
                            r0, r1 = HD * hh, HD * (hh + 1)
                            u_sb = sb.tile([HD, 512], F16, tag="u_sb", bufs=3,
                                           name="u_sb")
                            nc.vector.tensor_copy(u_sb[:], ot[hh][0:HD, :])
                            dr_f = sb.tile([1, 512], F32, tag="dr_f", bufs=2,
                                           name="dr_f")
                            nc.vector.tensor_copy(dr_f[:], ot[hh][64:65, :])
                            rr_f = sb.tile([1, 512], F32, tag="rr_f", bufs=2,
                                           name="rr_f")
                            nc.vector.reciprocal_approx_fast(out=rr_f[:],
                                                             in_=dr_f[:])
                            rr_h = sb.tile([1, 512], F16, tag="rr_h", bufs=2,
                                           name="rr_h")
                            nc.vector.tensor_copy(rr_h[:], rr_f[:])
                            rr_d = dp.tile([1, 512], F16, tag="rr_d", bufs=2,
                                           name="rr_d")
                            nc.sync.dma_start(rr_d[:], rr_h[:])
                            bc_h = sb.tile([HD, 512], F16, tag="bc_h", bufs=2,
                                           name="bc_h")
                            nc.sync.dma_start(bc_h[:],
                                              rr_d[:].to_broadcast((HD, 512)))
                            nc.vector.tensor_mul(at[jp][r0:r1, n0:n0 + 512],
                                                 u_sb[:], bc_h[:])

                # output projection (+ rank-1 bias), psum -> sbuf -> DRAM
                for t in range(8):
                    osb = sb.tile([P, D], F32, tag="outs", bufs=2, name="osb")
                    for c0, cw in ((0, 512), (512, 256)):
                        pps = ps.tile([P, 512], F32, tag="mm", bufs=2, name="pps")
                        for d in range(6):
                            nc.tensor.matmul(pps[:, 0:cw], at[d][:, P * t:P * (t + 1)],
                                             wp_h[d][:, c0:c0 + cw],
                                             start=(d == 0), stop=False)
                        nc.tensor.matmul(pps[:, 0:cw], ones_h[0:1, 0:P],
                                         bfinal_h[:, c0:c0 + cw],
                                         start=False, stop=True)
                        nc.vector.tensor_copy(osb[:, c0:c0 + cw], pps[:, 0:cw])
                    nc.sync.dma_start(out_il[b, t], osb[:])
    nc.compile()
    return nc


def _get_nc():
    if "nc" not in _CACHE:
        _CACHE["nc"] = _build()
    return _CACHE["nc"]


def kernel(x, W_qkv, b_qkv, W_proj, b_proj):
    from concourse.bass_utils import run_bass_kernel_spmd

    nc = _get_nc()
    x = np.ascontiguousarray(x, dtype=np.float32)
    in_maps = [
        {
            "x": x[2 * i:2 * i + 2],
            "W_qkv": np.asarray(W_qkv, dtype=np.float32),
            "b_qkv": np.asarray(b_qkv, dtype=np.float32),
            "W_proj": np.asarray(W_proj, dtype=np.float32),
            "b_proj": np.asarray(b_proj, dtype=np.float32),
        }
        for i in range(8)
    ]
    res = run_bass_kernel_spmd(nc, in_maps, core_ids=list(range(8)))
    return np.concatenate([r["out"] for r in res.results], axis=0)


# revision 25
# speedup vs baseline: 1.1308x; 1.0481x over previous
"""Multi-head self-attention (B=16, N=1024, D=768, H=12) on 8 TRN2 NeuronCores.

Data-parallel over batch (2 batches per core, weights replicated, no
collectives). Per core, one fused Bass/Tile kernel:

  x --one contiguous interleaved DMA--> x6 [128, 8*768] (token 8p+t on
      partition p, slot t; attention is permutation-invariant over tokens,
      so the interleave is only undone at the output DMA)
  x6 --f16 cast + PE transpose--> xT [d, tok]
  QT/KT = (W_qkv^T x^T + b) in [col, tok] layout (f16)
  V_aug = [x W_v | ones-col per head]  [tok, 12*65] (f16)
  per head: S^T[m,n] = K Q^T (PE), E = exp(S^T*scale) (ACT, [128,1024]),
      O^T = V_aug^T E (PE; row 64 = softmax denominator via the ones
      column -- no max subtraction needed, scores are O(1)).
      normalize: copy O^T/denom out of PSUM early, recip_approx_fast,
      DMA-broadcast the reciprocal row, one DVE mul.
  out = attnT^T W_proj + (W_proj^T b_v + b_proj)  (rank-1 bias matmul;
      V-bias folded through softmax since rows of A sum to 1)

All matmul operands f16 (1 cycle/row; fp32/f32r run 2-pass fp32_mode=HIGH
at 1/4 rate and break HAM warm-up -- measured). PSUM accumulation is f32.
"""

import numpy as np

_CACHE: dict = {}

P = 128
BL, N, D, H, HD = 2, 1024, 768, 12, 64
D3 = 3 * D
SCALE = float(HD) ** -0.5


def _build():
    import concourse.mybir as mybir
    import concourse.tile as tile
    from concourse import bacc
    from concourse.masks import make_identity

    dt = mybir.dt
    F32, F16 = dt.float32, dt.float16
    AF = mybir.ActivationFunctionType

    nc = bacc.Bacc("TRN2", target_bir_lowering=False, debug=False)
    x_d = nc.dram_tensor("x", [BL, N, D], F32, kind="ExternalInput").ap()
    wqkv_d = nc.dram_tensor("W_qkv", [D, D3], F32, kind="ExternalInput").ap()
    bqkv_d = nc.dram_tensor("b_qkv", [D3], F32, kind="ExternalInput").ap()
    wproj_d = nc.dram_tensor("W_proj", [D, D], F32, kind="ExternalInput").ap()
    bproj_d = nc.dram_tensor("b_proj", [D], F32, kind="ExternalInput").ap()
    out_d = nc.dram_tensor("out", [BL, N, D], F32, kind="ExternalOutput").ap()
    # token-interleaved views: partition p, slot t <-> token 8p+t
    x_il = x_d.rearrange("b (p i) d -> b p (i d)", p=P)       # [2, 128, 6144]
    out_il = out_d.rearrange("b (p i) d -> b i p d", p=P)     # [2, 8, 128, 768]

    with tile.TileContext(nc) as tc:
        with tc.tile_pool(name="sb", bufs=1) as sb, \
             tc.tile_pool(name="dp", bufs=1, space="DRAM") as dp, \
             tc.tile_pool(name="ps", bufs=2, space="PSUM") as ps:

            # ---------- constants ----------
            ident = sb.tile([P, P], F16, tag="ident", bufs=1, name="ident")
            make_identity(nc, ident[:])
            ones_h = sb.tile([P, P], F16, tag="ones_h", bufs=1, name="ones_h")
            nc.vector.memset(ones_h[:], 1.0)

            # ---------- PE warm-up: ~4us of dummy matmuls flips HAM to 8/8
            # (transposes run in transpose-mode, which does not warm HAM)
            warm_h = sb.tile([P, 512], F16, tag="e", bufs=3, name="warm_h")
            nc.vector.memset(warm_h[:], 0.0)
            for wi in range(10):
                wps = ps.tile([P, 512], F32, tag="mm", bufs=2, name="wps")
                nc.tensor.matmul(wps[:], ones_h[:, 0:P], warm_h[:],
                                 start=True, stop=True)

            # ---------- x load: one big interleaved DMA per batch ----------
            x6 = {}
            x6[0] = sb.tile([P, 8 * D], F32, tag="x6", bufs=1, name="x6")
            nc.sync.dma_start(x6[0][:, 0:4 * D], x_il[0][:, 0:4 * D])
            nc.sync.dma_start(x6[0][:, 4 * D:8 * D], x_il[0][:, 4 * D:8 * D])

            # ---------- W DMAs on the (idle) scalar queue, casts on DVE ----
            HW_ = D3 // 2
            wq_h, wp_h = [], []
            for d in range(6):
                t = sb.tile([P, D3], F16, tag=f"wqkv{d}", bufs=1, name=f"wqkv{d}")
                for half in range(2):
                    stg = sb.tile([P, HW_], F32, tag="wstage", bufs=2, name="wstg")
                    nc.scalar.dma_start(
                        stg[:], wqkv_d[P * d:P * (d + 1), HW_ * half:HW_ * (half + 1)])
                    nc.vector.tensor_copy(t[:, HW_ * half:HW_ * (half + 1)], stg[:])
                wq_h.append(t)

            xT = {b: [sb.tile([P, N], F16, tag=f"xT{b}_{j}", bufs=1,
                              name=f"xT{b}_{j}") for j in range(6)]
                  for b in range(BL)}

            def do_transposes(b):
                for t in range(8):
                    xh = sb.tile([P, D], F16, tag="xh", bufs=1, name="xh")
                    nc.vector.tensor_copy(xh[:], x6[b][:, D * t:D * (t + 1)])
                    for j in range(6):
                        tp = ps.tile([P, P], F16, tag="mm", bufs=2, name="tp")
                        nc.tensor.transpose(tp[:], xh[:, P * j:P * (j + 1)],
                                            ident[:])
                        nc.vector.tensor_copy(xT[b][j][:, P * t:P * (t + 1)],
                                              tp[:])

            do_transposes(0)

            for d in range(6):
                stg = sb.tile([P, HW_], F32, tag="wstage", bufs=2, name="wstg2")
                nc.scalar.dma_start(stg[:, 0:D], wproj_d[P * d:P * (d + 1), :])
                t = sb.tile([P, D], F16, tag=f"wproj{d}", bufs=1, name=f"wproj{d}")
                nc.vector.tensor_copy(t[:], stg[:, 0:D])
                wp_h.append(t)

            # ---------- biases (one [18,128] DMA + PE transpose) ----------
            bstg = sb.tile([18, P], F32, tag="bstg", bufs=1, name="bstg")
            nc.scalar.dma_start(bstg[:], bqkv_d.rearrange("(j p) -> j p", p=P))
            bstg_h = sb.tile([18, P], F16, tag="bstg_h", bufs=1, name="bstg_h")
            nc.vector.tensor_copy(bstg_h[:], bstg[:])
            btp = ps.tile([P, 18], F16, tag="mm", bufs=2, name="btp")
            nc.tensor.transpose(btp[:], bstg_h[:], ident[0:18, 0:18])
            bqkvT = sb.tile([P, 18], F32, tag="bqkvT", bufs=1, name="bqkvT")
            nc.vector.tensor_copy(bqkvT[:], btp[:])
            bv_h = sb.tile([P, 6], F16, tag="bv_h", bufs=1, name="bv_h")
            nc.vector.tensor_copy(bv_h[:], btp[:, 12:18])
            bproj_row = sb.tile([1, D], F32, tag="bproj_row", bufs=1, name="bproj_row")
            nc.scalar.dma_start(bproj_row[:], bproj_d.unsqueeze(0))

            # b_final = W_proj^T b_v + b_proj   [1, 768] f16
            bfinal_h = sb.tile([1, D], F16, tag="bfinal", bufs=1, name="bfinal")
            for c0, cw in ((0, 512), (512, 256)):
                bf_ps = ps.tile([1, 512], F32, tag="mm", bufs=2, name="bf_ps")
                for d in range(6):
                    nc.tensor.matmul(bf_ps[:, 0:cw], bv_h[:, d:d + 1],
                                     wp_h[d][:, c0:c0 + cw],
                                     start=(d == 0), stop=(d == 5))
                nc.vector.tensor_add(bfinal_h[:, c0:c0 + cw], bf_ps[0:1, 0:cw],
                                     bproj_row[:, c0:c0 + cw])

            # ---------- per-batch, with cross-phase interleaving ----------
            # Engines execute fixed in-order streams, so later-phase matmul
            # groups are emitted INSIDE the attention loop to fill PE slack
            # while ACT runs the exp stream.
            qk = {b: [sb.tile([P, N], F16, tag=f"qk{b%2}_{j}", bufs=1,
                              name=f"qk{j}") for j in range(12)]
                  for b in range(BL)}
            v = {b: [sb.tile([P, 12 * 65], F16, tag=f"v{b%2}_{t}", bufs=1,
                             name=f"v{t}") for t in range(8)]
                 for b in range(BL)}
            at = {b: [sb.tile([P, N], F16, tag=f"at{j}", bufs=1,
                              name=f"at{j}") for j in range(6)]
                  for b in range(BL)}

            def emit_qkv_group(b, j, nh):
                qps = ps.tile([P, 512], F32, tag="mm", bufs=2, name="qps")
                for d in range(6):
                    nc.tensor.matmul(qps[:], wq_h[d][:, P * j:P * (j + 1)],
                                     xT[b][d][:, 512 * nh:512 * (nh + 1)],
                                     start=(d == 0), stop=(d == 5))
                nc.vector.tensor_scalar_add(
                    qk[b][j][:, 512 * nh:512 * (nh + 1)], qps[:],
                    bqkvT[:, j:j + 1])

            def emit_v_group(b, t, ci):
                c0, cw = ((0, 512), (512, 256))[ci]
                v3 = v[b][t].rearrange("p (h c) -> p h c", c=65)
                if ci == 0:
                    nc.vector.tensor_copy(v3[:, :, 64:65],
                                          ones_h[:, 0:12].unsqueeze(2))
                vps = ps.tile([P, 512], F32, tag="mm", bufs=2, name="vps")
                for d in range(6):
                    nc.tensor.matmul(vps[:, 0:cw], xT[b][d][:, P * t:P * (t + 1)],
                                     wq_h[d][:, 2 * D + c0:2 * D + c0 + cw],
                                     start=(d == 0), stop=(d == 5))
                nc.vector.tensor_copy(
                    v3[:, (c0 // HD):((c0 + cw) // HD), 0:HD],
                    vps[:, 0:cw].rearrange("p (h c) -> p h c", c=HD))

            def emit_proj_group(b, t):
                osb = sb.tile([P, D], F32, tag="outs", bufs=2, name="osb")
                for c0, cw in ((0, 512), (512, 256)):
                    pps = ps.tile([P, 512], F32, tag="mm", bufs=2, name="pps")
                    for d in range(6):
                        nc.tensor.matmul(pps[:, 0:cw],
                                         at[b][d][:, P * t:P * (t + 1)],
                                         wp_h[d][:, c0:c0 + cw],
                                         start=(d == 0), stop=False)
                    nc.tensor.matmul(pps[:, 0:cw], ones_h[0:1, 0:P],
                                     bfinal_h[:, c0:c0 + cw],
                                     start=False, stop=True)
                    nc.vector.tensor_copy(osb[:, c0:c0 + cw], pps[:, 0:cw])
                nc.sync.dma_start(out_il[b, t], osb[:])

            def emit_attention(b, fillers):
                for jp in range(6):
                    qt, kt = qk[b][jp], qk[b][6 + jp]
                    for nh in range(2):
                        n0 = 512 * nh
                        ot = [ps.tile([65, 512], F32, tag="ot", bufs=2,
                                      name="otps") for _ in range(2)]
                        pend = []
                        for m in range(8):
                            sps = ps.tile([P, N], F32, tag="s", bufs=2,
                                          name="sps")
                            for hh in range(2):
                                r0, r1 = HD * hh, HD * (hh + 1)
                                nc.tensor.matmul(
                                    sps[:, 512 * hh:512 * (hh + 1)],
                                    kt[r0:r1, P * m:P * (m + 1)],
                                    qt[r0:r1, n0:n0 + 512],
                                    start=True, stop=True)
                            e = sb.tile([P, N], F16, tag="e", bufs=3, name="e")
                            nc.scalar.activation(e[:], sps[:], AF.Exp,
                                                 scale=SCALE)
                            pend.append((m, e))
                            if len(pend) == 2:
                                pm, pe_ = pend.pop(0)
                                for hh in range(2):
                                    h = 2 * jp + hh
                                    nc.tensor.matmul(
                                        ot[hh][:],
                                        v[b][pm][:, 65 * h:65 * h + 65],
                                        pe_[:, 512 * hh:512 * (hh + 1)],
                                        start=(pm == 0), stop=(pm == 7))
                            if m in (2, 5) and fillers:
                                fillers.pop(0)()
                        for pm, pe_ in pend:
                            for hh in range(2):
                                h = 2 * jp + hh
                                nc.tensor.matmul(
                                    ot[hh][:], v[b][pm][:, 65 * h:65 * h + 65],
                                    pe_[:, 512 * hh:512 * (hh + 1)],
                                    start=(pm == 0), stop=(pm == 7))
                        for hh in range(2):
                            r0, r1 = HD * hh, HD * (hh + 1)
                            u_sb = sb.tile([HD, 512], F16, tag="u_sb", bufs=2,
                                           name="u_sb")
                            nc.vector.tensor_copy(u_sb[:], ot[hh][0:HD, :])
                            dr_f = sb.tile([1, 512], F32, tag="dr_f", bufs=1,
                                           name="dr_f")
                            nc.vector.tensor_copy(dr_f[:], ot[hh][64:65, :])
                            rr_f = sb.tile([1, 512], F32, tag="rr_f", bufs=1,
                                           name="rr_f")
                            nc.vector.reciprocal_approx_fast(out=rr_f[:],
                                                             in_=dr_f[:])
                            rr_h = sb.tile([1, 512], F16, tag="rr_h", bufs=2,
                                           name="rr_h")
                            nc.vector.tensor_copy(rr_h[:], rr_f[:])
                            rr_d = dp.tile([1, 512], F16, tag="rr_d", bufs=2,
                                           name="rr_d")
                            nc.sync.dma_start(rr_d[:], rr_h[:])
                            bc_h = sb.tile([HD, 512], F16, tag="bc_h", bufs=2,
                                           name="bc_h")
                            nc.sync.dma_start(bc_h[:],
                                              rr_d[:].to_broadcast((HD, 512)))
                            nc.vector.tensor_mul(at[b][jp][r0:r1, n0:n0 + 512],
                                                 u_sb[:], bc_h[:])
                        if fillers:
                            fillers.pop(0)()

            # batch 0 projections (serial -- nothing to overlap with yet)
            for j in range(12):
                for nh in range(2):
                    emit_qkv_group(0, j, nh)
            for t in range(8):
                for ci in range(2):
                    emit_v_group(0, t, ci)

            # batch 1 x load + transposes before batch-0 attention
            x6[1] = sb.tile([P, 8 * D], F32, tag="x6", bufs=1, name="x6")
            nc.sync.dma_start(x6[1][:, 0:4 * D], x_il[1][:, 0:4 * D])
            nc.sync.dma_start(x6[1][:, 4 * D:8 * D], x_il[1][:, 4 * D:8 * D])
            do_transposes(1)

            # batch-0 attention with batch-1 QKV/V interleaved
            fill0 = [lambda j=j, nh=nh: emit_qkv_group(1, j, nh)
                     for j in range(12) for nh in range(2)]
            fill0 += [lambda t=t, ci=ci: emit_v_group(1, t, ci)
                      for t in range(8) for ci in range(2)]
            emit_attention(0, fill0)
            for f in fill0:
                f()

            # batch-1 attention with batch-0 projection interleaved
            fill1 = [lambda t=t: emit_proj_group(0, t) for t in range(8)]
            emit_attention(1, fill1)
            for f in fill1:
                f()

            # batch-1 projection (tail)
            for t in range(8):
                emit_proj_group(1, t)
    nc.compile()
    return nc


def _get_nc():
    if "nc" not in _CACHE:
        _CACHE["nc"] = _build()
    return _CACHE["nc"]


def kernel(x, W_qkv, b_qkv, W_proj, b_proj):
    from concourse.bass_utils import run_bass_kernel_spmd

    nc = _get_nc()
    x = np.ascontiguousarray(x, dtype=np.float32)
    in_maps = [
        {
            "x": x[2 * i:2 * i + 2],
            "W_qkv": np.asarray(W_qkv, dtype=np.float32),
            "b_qkv": np.asarray(b_qkv, dtype=np.float32),
            "W_proj": np.asarray(W_proj, dtype=np.float32),
            "b_proj": np.asarray(b_proj, dtype=np.float32),
        }
        for i in range(8)
    ]
    res = run_bass_kernel_spmd(nc, in_maps, core_ids=list(range(8)))
    return np.concatenate([r["out"] for r in res.results], axis=0)
